# revision 1
# baseline (speedup 1.0000x reference)
"""CTC greedy decode kernel for Trainium2 (8 NeuronCores, data-parallel over batch).

Reference computation (per batch row b):
  best[t]  = argmax_c probs[b, t, c]          (first index wins ties)
  valid[t] = (best[t] != best[t-1]) & (best[t] != C-1)
  left-pack best[valid] -> slots 0..cnt-1, map through table, pad with default.

Device algorithm (b on partitions, 128 rows per core):
  For each t: the argmax value AND its table char are extracted with one
  fused encoding: enc[c] = (127-c)*1024 + table[c] (fits exactly in fp32).
    vmax = max_c v                     (exact fp32 compare)
    z    = v - vmax                    (<= 0, == 0 exactly at maxima)
    mi   = z * 2^44 + enc[c]           (< 0 wherever z != 0; == enc at maxima)
    kres = max_c mi = (127-c*)*1024 + table[c*],  c* = FIRST argmax index
  kres doubles as a collapsed label id (equality in kres-space == equality in
  label-space; kres == table[127] iff label == blank).  chars = low 10 bits of
  kres.  The left-pack is a gpsimd local_scatter with cumsum-derived slots
  (invalid positions get index -1, which local_scatter ignores); empty slots
  are then filled with default_char via an iota/count mask.
"""

import os
import sys

sys.path.insert(0, "/opt/trn_rl_repo")

import numpy as np

import concourse.bacc as bacc
import concourse.bass as bass
import concourse.mybir as mybir
from concourse import bass_utils
from concourse.tile import TileContext

B, T, C = 1024, 512, 128
NCORES = 8
BL = B // NCORES  # 128 batch rows per core == partition count
TC = 32           # timesteps per chunk
NCHUNK = T // TC
BIG = float(2 ** 44)
F32 = mybir.dt.float32
I32 = mybir.dt.int32
I16 = mybir.dt.int16
ALU = mybir.AluOpType
AX = mybir.AxisListType


def build_module():
    nc = bacc.Bacc("TRN2", target_bir_lowering=False, debug=False)

    x = nc.dram_tensor("x", [BL, T, C], F32, kind="ExternalInput")
    enc_d = nc.dram_tensor("enc", [128, C], F32, kind="ExternalInput")
    iota_d = nc.dram_tensor("iota_t", [128, T], F32, kind="ExternalInput")
    blank_d = nc.dram_tensor("blankk", [128, 1], F32, kind="ExternalInput")
    dflt_d = nc.dram_tensor("dflt", [128, 1], F32, kind="ExternalInput")
    y = nc.dram_tensor("y", [BL, T], I32, kind="ExternalOutput")

    with TileContext(nc) as tc:
        with (
            tc.tile_pool(name="consts", bufs=1) as cpool,
            tc.tile_pool(name="vp", bufs=3) as vpool,
            tc.tile_pool(name="zp", bufs=2) as zpool,
            tc.tile_pool(name="mp", bufs=2) as mpool,
            tc.tile_pool(name="small", bufs=1) as spool,
        ):
            enc_t = cpool.tile([128, C], F32, tag="enc")
            nc.sync.dma_start(enc_t[:], enc_d.ap())
            iota_t = cpool.tile([128, T], F32, tag="iota")
            nc.sync.dma_start(iota_t[:], iota_d.ap())
            blank_t = cpool.tile([128, 1], F32, tag="blank")
            nc.sync.dma_start(blank_t[:], blank_d.ap())
            dflt_t = cpool.tile([128, 1], F32, tag="dflt")
            nc.sync.dma_start(dflt_t[:], dflt_d.ap())
            zeros_t = cpool.tile([128, T], F32, tag="zeros")
            nc.vector.memset(zeros_t[:], 0.0)

            kres = spool.tile([128, T], F32, tag="kres")
            vmax = spool.tile([128, T], F32, tag="vmax")

            for i in range(NCHUNK):
                sl = bass.ts(i, TC)
                v = vpool.tile([128, TC * C], F32, tag="v")
                nc.sync.dma_start(v[:], x.ap()[:, sl, :])
                v3 = v[:].rearrange("p (t c) -> p t c", c=C)

                vm = vmax[:, sl]
                nc.vector.tensor_reduce(vm, v3, axis=AX.X, op=ALU.max)

                z = zpool.tile([128, TC * C], F32, tag="z")
                z3 = z[:].rearrange("p (t c) -> p t c", c=C)
                vmb = vm.unsqueeze(2).broadcast_to([128, TC, C])
                nc.vector.tensor_tensor(z3, v3, vmb, op=ALU.subtract)

                mi = mpool.tile([128, TC * C], F32, tag="mi")
                mi3 = mi[:].rearrange("p (t c) -> p t c", c=C)
                encb = enc_t[:].unsqueeze(1).broadcast_to([128, TC, C])
                nc.vector.scalar_tensor_tensor(
                    mi3, z3, BIG, encb, op0=ALU.mult, op1=ALU.add
                )
                nc.vector.tensor_reduce(kres[:, sl], mi3, axis=AX.X, op=ALU.max)

            # chars = kres mod 1024, via hi = int(kres/1024) (frac < 0.5 so any
            # rounding mode truncates correctly), chars = kres - 1024*hi.
            hi_i = spool.tile([128, T], I32, tag="hi")
            nc.vector.tensor_scalar_mul(hi_i[:], kres[:], 1.0 / 1024.0)
            chars = spool.tile([128, T], F32, tag="chars")
            nc.vector.scalar_tensor_tensor(
                chars[:], hi_i[:], -1024.0, kres[:], op0=ALU.mult, op1=ALU.add
            )

            # previous label (kres-space), with -1 sentinel in column 0
            kprev = spool.tile([128, T], F32, tag="kprev")
            nc.vector.memset(kprev[:, 0:1], -1.0)
            nc.vector.tensor_copy(kprev[:, 1:T], kres[:, 0 : T - 1])

            neq = spool.tile([128, T], F32, tag="neq")
            nc.vector.tensor_tensor(neq[:], kres[:], kprev[:], op=ALU.not_equal)
            valid = spool.tile([128, T], F32, tag="valid")
            nc.vector.scalar_tensor_tensor(
                valid[:], kres[:], blank_t[:, 0:1], neq[:],
                op0=ALU.not_equal, op1=ALU.mult,
            )

            csum = spool.tile([128, T], F32, tag="csum")
            nc.vector.tensor_tensor_scan(
                csum[:], valid[:], zeros_t[:], 0.0, op0=ALU.add, op1=ALU.add
            )
            cnt = csum[:, T - 1 : T]

            pv = spool.tile([128, T], F32, tag="pv")
            nc.vector.tensor_tensor(pv[:], csum[:], valid[:], op=ALU.mult)
            scol = spool.tile([128, T], F32, tag="scol")
            nc.vector.tensor_scalar_add(scol[:], pv[:], -1.0)

            scol_i = spool.tile([128, T], I16, tag="scol_i")
            nc.vector.tensor_copy(scol_i[:], scol[:])
            chars_i = spool.tile([128, T], I16, tag="chars_i")
            nc.vector.tensor_copy(chars_i[:], chars[:])

            packed = spool.tile([128, T], I16, tag="packed")
            nc.gpsimd.local_scatter(
                packed[:], chars_i[:], scol_i[:],
                channels=128, num_elems=T, num_idxs=T,
            )

            m1 = spool.tile([128, T], F32, tag="m1")
            nc.vector.scalar_tensor_tensor(
                m1[:], iota_t[:], cnt, packed[:], op0=ALU.is_lt, op1=ALU.mult
            )
            m2 = spool.tile([128, T], F32, tag="m2")
            dfb = dflt_t[:, 0:1].broadcast_to([128, T])
            nc.vector.scalar_tensor_tensor(
                m2[:], iota_t[:], cnt, dfb, op0=ALU.is_ge, op1=ALU.mult
            )
            out_t = spool.tile([128, T], I32, tag="out")
            nc.vector.tensor_tensor(out_t[:], m1[:], m2[:], op=ALU.add)

            nc.sync.dma_start(y.ap(), out_t[:])

    nc.compile()
    return nc


def make_const_inputs(table: np.ndarray, default_char) -> dict[str, np.ndarray]:
    table = np.asarray(table).astype(np.int64)
    enc_row = ((127 - np.arange(C, dtype=np.int64)) * 1024 + table).astype(np.float32)
    return {
        "enc": np.tile(enc_row, (128, 1)),
        "iota_t": np.tile(np.arange(T, dtype=np.float32), (128, 1)),
        "blankk": np.full((128, 1), float(table[C - 1]), np.float32),
        "dflt": np.full((128, 1), float(default_char), np.float32),
    }


_NC_CACHE = None


def kernel(inputs, table, default_char):
    global _NC_CACHE
    inputs = np.ascontiguousarray(np.asarray(inputs, dtype=np.float32))
    table_np = np.asarray(table)
    assert inputs.shape == (B, T, C), inputs.shape

    if _NC_CACHE is None:
        _NC_CACHE = build_module()
    nc = _NC_CACHE

    consts = make_const_inputs(table_np, default_char)
    in_maps = []
    for i in range(NCORES):
        m = {"x": inputs[i * BL : (i + 1) * BL]}
        m.update(consts)
        in_maps.append(m)

    res = bass_utils.run_bass_kernel_spmd(nc, in_maps, core_ids=list(range(NCORES)))
    out = np.concatenate([r["y"] for r in res.results], axis=0)
    return out.astype(np.int32)


if __name__ == "__main__":
    import reference

    inp = reference.setup_inputs()
    out = kernel(**{k: np.asarray(v) for k, v in inp.items()})
    print(out.shape, out.dtype)


# revision 3
# speedup vs baseline: 104561.0768x; 104561.0768x over previous
"""CTC greedy decode kernel for Trainium2 (8 NeuronCores, data-parallel over batch).

Reference computation (per batch row b):
  best[t]  = argmax_c probs[b, t, c]          (first index wins ties)
  valid[t] = (best[t] != best[t-1]) & (best[t] != C-1)
  left-pack best[valid] -> slots 0..cnt-1, map through table, pad with default.

Device algorithm (b on partitions, 128 rows per core):
  For each t: the argmax value AND its table char are extracted with one
  fused encoding: enc[c] = (127-c)*1024 + table[c] (fits exactly in fp32).
    vmax = max_c v                     (exact fp32 compare)
    z    = v - vmax                    (<= 0, == 0 exactly at maxima)
    mi   = z * 2^44 + enc[c]           (< 0 wherever z != 0; == enc at maxima)
    kres = max_c mi = (127-c*)*1024 + table[c*],  c* = FIRST argmax index
  kres doubles as a collapsed label id (equality in kres-space == equality in
  label-space; kres == table[127] iff label == blank).  chars = low 10 bits of
  kres.  The left-pack is a gpsimd local_scatter with cumsum-derived slots
  (invalid positions get index -1, which local_scatter ignores); empty slots
  are then filled with default_char via an iota/count mask.
"""

import os
import sys

sys.path.insert(0, "/opt/trn_rl_repo")

import numpy as np

import concourse.bacc as bacc
import concourse.bass as bass
import concourse.mybir as mybir
from concourse import bass_utils
from concourse.tile import TileContext

B, T, C = 1024, 512, 128
NCORES = 8
BL = B // NCORES  # 128 batch rows per core == partition count
TC = 32           # timesteps per chunk
NCHUNK = T // TC
BIG = float(2 ** 44)
F32 = mybir.dt.float32
I32 = mybir.dt.int32
I16 = mybir.dt.int16
ALU = mybir.AluOpType
AX = mybir.AxisListType


def build_module():
    nc = bacc.Bacc("TRN2", target_bir_lowering=False, debug=False)

    x = nc.dram_tensor("x", [BL, T, C], F32, kind="ExternalInput")
    enc_d = nc.dram_tensor("enc", [128, C], F32, kind="ExternalInput")
    iota_d = nc.dram_tensor("iota_t", [128, T], F32, kind="ExternalInput")
    blank_d = nc.dram_tensor("blankk", [128, 1], F32, kind="ExternalInput")
    dflt_d = nc.dram_tensor("dflt", [128, 1], F32, kind="ExternalInput")
    y = nc.dram_tensor("y", [BL, T], I32, kind="ExternalOutput")

    with TileContext(nc) as tc:
        with (
            tc.tile_pool(name="consts", bufs=1) as cpool,
            tc.tile_pool(name="vp", bufs=3) as vpool,
            tc.tile_pool(name="zp", bufs=2) as zpool,
            tc.tile_pool(name="mp", bufs=2) as mpool,
            tc.tile_pool(name="small", bufs=1) as spool,
        ):
            enc_t = cpool.tile([128, C], F32, tag="enc")
            nc.sync.dma_start(enc_t[:], enc_d.ap())
            iota_t = cpool.tile([128, T], F32, tag="iota")
            nc.sync.dma_start(iota_t[:], iota_d.ap())
            blank_t = cpool.tile([128, 1], F32, tag="blank")
            nc.sync.dma_start(blank_t[:], blank_d.ap())
            dflt_t = cpool.tile([128, 1], F32, tag="dflt")
            nc.sync.dma_start(dflt_t[:], dflt_d.ap())
            zeros_t = cpool.tile([128, T], F32, tag="zeros")
            nc.vector.memset(zeros_t[:], 0.0)

            kres = spool.tile([128, T], F32, tag="kres")
            vmax = spool.tile([128, T], F32, tag="vmax")

            for i in range(NCHUNK):
                sl = bass.ts(i, TC)
                v = vpool.tile([128, TC * C], F32, tag="v")
                nc.sync.dma_start(v[:], x.ap()[:, sl, :])
                v3 = v[:].rearrange("p (t c) -> p t c", c=C)

                vm = vmax[:, sl]
                nc.vector.tensor_reduce(vm, v3, axis=AX.X, op=ALU.max)

                z = zpool.tile([128, TC * C], F32, tag="z")
                z3 = z[:].rearrange("p (t c) -> p t c", c=C)
                vmb = vm.unsqueeze(2).broadcast_to([128, TC, C])
                nc.vector.tensor_tensor(z3, v3, vmb, op=ALU.subtract)

                mi = mpool.tile([128, TC * C], F32, tag="mi")
                mi3 = mi[:].rearrange("p (t c) -> p t c", c=C)
                encb = enc_t[:].unsqueeze(1).broadcast_to([128, TC, C])
                nc.vector.scalar_tensor_tensor(
                    mi3, z3, BIG, encb, op0=ALU.mult, op1=ALU.add
                )
                nc.vector.tensor_reduce(kres[:, sl], mi3, axis=AX.X, op=ALU.max)

            # chars = kres mod 1024, via hi = int(kres/1024) (frac < 0.5 so any
            # rounding mode truncates correctly), chars = kres - 1024*hi.
            hi_i = spool.tile([128, T], I32, tag="hi")
            nc.vector.tensor_scalar_mul(hi_i[:], kres[:], 1.0 / 1024.0)
            chars = spool.tile([128, T], F32, tag="chars")
            nc.vector.scalar_tensor_tensor(
                chars[:], hi_i[:], -1024.0, kres[:], op0=ALU.mult, op1=ALU.add
            )

            # previous label (kres-space), with -1 sentinel in column 0
            kprev = spool.tile([128, T], F32, tag="kprev")
            nc.vector.memset(kprev[:, 0:1], -1.0)
            nc.vector.tensor_copy(kprev[:, 1:T], kres[:, 0 : T - 1])

            neq = spool.tile([128, T], F32, tag="neq")
            nc.vector.tensor_tensor(neq[:], kres[:], kprev[:], op=ALU.not_equal)
            valid = spool.tile([128, T], F32, tag="valid")
            nc.vector.scalar_tensor_tensor(
                valid[:], kres[:], blank_t[:, 0:1], neq[:],
                op0=ALU.not_equal, op1=ALU.mult,
            )

            csum = spool.tile([128, T], F32, tag="csum")
            nc.vector.tensor_tensor_scan(
                csum[:], valid[:], zeros_t[:], 0.0, op0=ALU.add, op1=ALU.add
            )
            cnt = csum[:, T - 1 : T]

            pv = spool.tile([128, T], F32, tag="pv")
            nc.vector.tensor_tensor(pv[:], csum[:], valid[:], op=ALU.mult)
            scol = spool.tile([128, T], F32, tag="scol")
            nc.vector.tensor_scalar_add(scol[:], pv[:], -1.0)

            scol_i = spool.tile([128, T], I16, tag="scol_i")
            nc.vector.tensor_copy(scol_i[:], scol[:])
            chars_i = spool.tile([128, T], I16, tag="chars_i")
            nc.vector.tensor_copy(chars_i[:], chars[:])

            packed = spool.tile([128, T], I16, tag="packed")
            nc.gpsimd.local_scatter(
                packed[:], chars_i[:], scol_i[:],
                channels=128, num_elems=T, num_idxs=T,
            )

            m1 = spool.tile([128, T], F32, tag="m1")
            nc.vector.scalar_tensor_tensor(
                m1[:], iota_t[:], cnt, packed[:], op0=ALU.is_lt, op1=ALU.mult
            )
            m2 = spool.tile([128, T], F32, tag="m2")
            dfb = dflt_t[:, 0:1].broadcast_to([128, T])
            nc.vector.scalar_tensor_tensor(
                m2[:], iota_t[:], cnt, dfb, op0=ALU.is_ge, op1=ALU.mult
            )
            out_t = spool.tile([128, T], I32, tag="out")
            nc.vector.tensor_tensor(out_t[:], m1[:], m2[:], op=ALU.add)

            nc.sync.dma_start(y.ap(), out_t[:])

    nc.compile()
    return nc


def make_const_inputs(table: np.ndarray, default_char) -> dict[str, np.ndarray]:
    table = np.asarray(table).astype(np.int64)
    enc_row = ((127 - np.arange(C, dtype=np.int64)) * 1024 + table).astype(np.float32)
    return {
        "enc": np.tile(enc_row, (128, 1)),
        "iota_t": np.tile(np.arange(T, dtype=np.float32), (128, 1)),
        "blankk": np.full((128, 1), float(table[C - 1]), np.float32),
        "dflt": np.full((128, 1), float(default_char), np.float32),
    }


_NC_CACHE = None
_JIT_CACHE = None


def _get_jit():
    """Build the bass module once and wrap it in a cached jit(shard_map(...))
    across the 8 cores, mirroring bass2jax.run_bass_via_pjrt but reusable
    across calls (no per-call retrace/recompile)."""
    global _NC_CACHE, _JIT_CACHE
    if _JIT_CACHE is not None:
        return _JIT_CACHE

    import jax
    from jax.sharding import Mesh, PartitionSpec
    try:
        from jax.experimental.shard_map import shard_map
    except ImportError:  # newer jax
        from jax.shard_map import shard_map
    from concourse import bass2jax

    if _NC_CACHE is None:
        _NC_CACHE = build_module()
    nc = _NC_CACHE

    bass2jax.install_neuronx_cc_hook()

    partition_name = (
        nc.partition_id_tensor.name if nc.partition_id_tensor else None
    )
    in_names: list[str] = []
    out_names: list[str] = []
    out_avals = []
    zero_outs: list[np.ndarray] = []
    for alloc in nc.m.functions[0].allocations:
        if not isinstance(alloc, mybir.MemoryLocationSet):
            continue
        name = alloc.memorylocations[0].name
        if alloc.kind == "ExternalInput":
            if name != partition_name:
                in_names.append(name)
        elif alloc.kind == "ExternalOutput":
            shape = tuple(alloc.tensor_shape)
            dtype = mybir.dt.np(alloc.dtype)
            out_names.append(name)
            out_avals.append(jax.core.ShapedArray(shape, dtype))
            zero_outs.append(np.zeros(shape, dtype))
    n_params = len(in_names)
    all_names = in_names + out_names
    if partition_name is not None:
        all_names = all_names + [partition_name]

    def _body(*args):
        operands = list(args)
        if partition_name is not None:
            operands.append(bass2jax.partition_id_tensor())
        outs = bass2jax._bass_exec_p.bind(
            *operands,
            out_avals=tuple(out_avals),
            in_names=tuple(all_names),
            out_names=tuple(out_names),
            lowering_input_output_aliases=(),
            sim_require_finite=True,
            sim_require_nnan=True,
            nc=nc,
        )
        return tuple(outs)

    devices = jax.devices()[:NCORES]
    mesh = Mesh(np.asarray(devices), ("core",))
    n_outs = len(out_names)
    sharded = jax.jit(
        shard_map(
            _body,
            mesh=mesh,
            in_specs=(PartitionSpec("core"),) * (n_params + n_outs),
            out_specs=(PartitionSpec("core"),) * n_outs,
            check_rep=False,
        ),
        keep_unused=True,
    )
    _JIT_CACHE = (sharded, in_names, out_names, zero_outs, mesh)
    return _JIT_CACHE


def _global_inputs(inputs: np.ndarray, table: np.ndarray, default_char):
    """Concatenated (8*per_core_shape[0], ...) global arrays, keyed by name."""
    consts = make_const_inputs(table, default_char)
    g = {"x": inputs}  # [1024, T, C] == concat of 8 x [128, T, C]
    for k, v in consts.items():
        g[k] = np.concatenate([v] * NCORES, axis=0)
    return g


def kernel(inputs, table, default_char):
    inputs = np.ascontiguousarray(np.asarray(inputs, dtype=np.float32))
    table_np = np.asarray(table)
    assert inputs.shape == (B, T, C), inputs.shape

    sharded, in_names, out_names, zero_outs, mesh = _get_jit()
    g = _global_inputs(inputs, table_np, default_char)
    args = [g[n] for n in in_names] + [
        np.zeros((NCORES * z.shape[0], *z.shape[1:]), z.dtype) for z in zero_outs
    ]
    out_arrs = sharded(*args)
    out = np.asarray(out_arrs[out_names.index("y")])
    return out.astype(np.int32)


if __name__ == "__main__":
    import reference

    inp = reference.setup_inputs()
    out = kernel(**{k: np.asarray(v) for k, v in inp.items()})
    print(out.shape, out.dtype)
